# revision 48
# baseline (speedup 1.0000x reference)
"""BuzzLoss Trainium2 kernel.

Math (telescoped + Horner form of the reference):
    excl[t] = prod_{s<t} (1 - conf[s])          (exclusive cumprod)
    da[0]   = 0 (the t=0 term, = acc[b,0], is added by the host)
    da[t]   = acc[t] - acc[t-1]
    score_b = acc[b,0] + sum_{t>=1} excl[t]*da[t]
            = acc[b,0] + nb_0*(da[1] + nb_1*(da[2] + ... ))   [Horner]
    out     = -mean_b score_b

The Horner bracket is one affine recurrence B_t = da[t] + nb_t * B_{t+1},
evaluated by a single DVE tensor_tensor_scan (op0=mult, op1=add) over the
REVERSED nb/da streams (negative-stride APs); the scan's final element is
the whole per-row score sum.  No cumprod tensor, no multiply+reduce pass.

Engine facts measured on this part (slope-based, per [128,1024] block):
    DVE scan ~2.0us (recurrence-limited), DVE elementwise ~0.77us,
    ACT ~0.95us, Pool tensor_sub ~1.9us,
    DVE and Pool FULLY SERIALIZE (no concurrency) -> Pool left idle,
    one HWDGE ring sustains the full ~360 GB/s -> all DMAs on SP,
    DMA floor for the 8 MiB/core inputs ~23.2us/rep.
Budget/rep: DVE = 5 subs + 8 scans + zeroing ~23us; ACT = nb + result
extraction ~8us; SP = DMA.  Steady state ~ the DMA floor.

Sharding: pure data parallel - batch 8192 split across 8 NeuronCores (1024
rows each).  Each core emits one f32 column per 128-row block; the host
combines, adds the t=0 boundary terms, takes the mean, negates.  No
collectives.

Chunking: chunk c covers m*128 consecutive DRAM rows as "(p j) t -> p (j t)"
so each partition holds m whole rows (m*4KiB contiguous DMA descriptors).
Per chunk:
    ACT : nb = 1 - conf, whole chunk (1 instr)
    DVE : da[1:] = acc[1:] - acc[:-1], whole chunk (1 instr); the m
          block-seam/da[0] columns are then overwritten with acc[row
          start] ([P,1] ACT copies) - the cross-row garbage positions
          are exactly the columns where Horner folds the t=0 term
    DVE : per block, the reversed affine scan, written into a SLIDING
          window of one shared scratch tile so every block's final
          element (its score) survives at a distinct column; the out-DMA
          reads the NBLK score columns directly (no extraction pass)
"""

import numpy as np

import concourse.bacc as bacc
import concourse.mybir as mybir
import concourse.tile as tile
from concourse.bass_utils import run_bass_kernel_spmd

B, T = 8192, 1024
N_CORES = 8
ROWS = B // N_CORES  # rows per core
P = 128  # SBUF partitions
NBLK = ROWS // P  # 1024-col blocks per core

# rows-per-partition per chunk; sum == NBLK.  Big chunks early (fewer
# DMAs), small chunks last (shorter single-shot tail).
CHUNKS = (2, 2, 2, 1, 1)

f32 = mybir.dt.float32
bf16 = mybir.dt.bfloat16

_CACHE = {}

NCOLS = NBLK


def _chunk_view(param, r0, m):
    """DRAM AP for m*P consecutive rows starting at row r0: [P, m*T],
    partition p holds m consecutive DRAM rows (m*4KiB contiguous)."""
    k = r0 // (m * P)
    assert k * m * P == r0, (r0, m)
    v = param.rearrange("(k p j) t -> k p (j t)", p=P, j=m)
    return v[k]


def _emit_pipeline(
    nc,
    io_pool,
    work_pool,
    res,
    conf,
    acc,
    rep,
    chunks,
    mode="full",
    io_tiles=None,
    rings="sp",
    grouped=True,
):
    Alu = mybir.AluOpType
    acc_dma = {"split": nc.scalar, "sp": nc.sync, "pool": nc.gpsimd}[rings]

    # ---- loads: per-chunk large DMAs ----
    if mode == "computeonly" and rep > 0:
        conf_ct, acc_ct = io_tiles
    else:
        conf_ct, acc_ct = [], []
        r0 = 0
        for ci, m in enumerate(chunks):
            pool = io_pool[m]
            ct = pool.tile([P, m * T], f32, tag=f"conf{m}", name=f"conf_r{rep}_c{ci}")
            at = pool.tile([P, m * T], f32, tag=f"acc{m}", name=f"acc_r{rep}_c{ci}")
            nc.sync.dma_start(ct[:], _chunk_view(conf, r0, m))
            acc_dma.dma_start(at[:], _chunk_view(acc, r0, m))
            conf_ct.append(ct)
            acc_ct.append(at)
            r0 += m * P
        if io_tiles is not None:
            io_tiles[0][:] = conf_ct
            io_tiles[1][:] = acc_ct

    if mode == "dmaonly":
        return

    # ---- compute ----
    # Phase 1 (per chunk): nb = 1-conf (ACT); da[i>=1] = acc[i]-acc[i-1]
    # (DVE, whole chunk); da[block starts] := acc[block starts] (ACT) -
    # the Horner step for t=0 has coefficient excl[0]=1, folding the t=0
    # boundary term (reference's correction) into the scan on-device.
    # Phase 2: the reversed affine scans (DVE).
    # Phase 3: per-block result-column extraction (ACT).
    # grouped=True batches each phase across all chunks (uniform DVE
    # instruction stream, best steady-state rate when reps are pipelined);
    # grouped=False interleaves per chunk so a single-shot execution's DVE
    # stream never waits on a later chunk's DMA (best reps=1 latency).
    scans = []
    blk = 0
    for ci, m in enumerate(chunks):
        conf_t = conf_ct[ci]
        acc_t = acc_ct[ci]
        W = m * T
        nb = work_pool.tile([P, W], f32, tag=f"nb{m}")
        nc.scalar.activation(
            nb[:],
            conf_t[:],
            mybir.ActivationFunctionType.Copy,
            bias=1.0,
            scale=-1.0,
        )
        da = work_pool.tile([P, W], f32, tag=f"da{m}")
        nc.vector.scalar_tensor_tensor(
            da[:, 1:W],
            acc_t[:, 1:W],
            1.0,
            acc_t[:, : W - 1],
            Alu.bypass,
            Alu.subtract,
        )
        # da[row starts] := acc[row starts], one strided op per chunk:
        # the Horner step for t=0 has coefficient excl[0]=1, folding the
        # t=0 boundary term into the scan on-device
        nc.scalar.activation(
            da[:, 0::T],
            acc_t[:, 0::T],
            mybir.ActivationFunctionType.Copy,
        )
        for j in range(m):
            scans.append((nb, da, j * T, blk))
            blk += 1
        if not grouped:
            _emit_scans(nc, res, scans)
            scans = []
    if grouped:
        _emit_scans(nc, res, scans)


def _emit_scans(nc, X, scans):
    # All scans share one sliding scratch window X [P, T+NBLK-1]:
    # scan for block b writes X[:, NBLK-1-b : NBLK-1-b+T], so its final
    # element (the per-row score) lands at col T+NBLK-2-b.  Later scans'
    # windows slide LEFT, so no later scan touches an earlier score:
    # the NBLK scores accumulate at cols [T-1, T+NBLK-1) (in reverse
    # block order — irrelevant, the host sums them) and the final
    # out-DMA reads them straight from X.  No per-block extraction, no
    # DVE->ACT->DVE scratch round-trip.  The overlapping writes are WAW
    # on one engine (DVE), which only orders them.
    Alu = mybir.AluOpType
    for nb, da, c0, blk_ in scans:
        s0 = NBLK - 1 - blk_
        # reversed affine scan: state = nb_rev*state + da_rev;
        # final element = the complete per-row score
        nc.vector.tensor_tensor_scan(
            X[:, s0 : s0 + T],
            nb[:, c0 : c0 + T][:, ::-1],
            da[:, c0 : c0 + T][:, ::-1],
            0.0,
            Alu.mult,
            Alu.add,
        )


def build_bass(reps: int = 1, chunks=CHUNKS, mode="full", rings="sp", bufs=(5, 3, 4)):
    iob2, iob1, workb = bufs
    nc = bacc.Bacc("TRN2", target_bir_lowering=False, debug=False)
    conf = nc.declare_dram_parameter("confidences", [ROWS, T], f32, isOutput=False)
    acc = nc.declare_dram_parameter("accuracies", [ROWS, T], f32, isOutput=False)
    out = nc.declare_dram_parameter("partials", [P, NCOLS], f32, isOutput=True)

    from contextlib import ExitStack

    with tile.TileContext(nc) as tc, ExitStack() as stack:
        io_bufs = {2: iob2, 1: iob1}
        io_pool = {
            m: stack.enter_context(
                tc.tile_pool(
                    name=f"io{m}",
                    bufs=(
                        chunks.count(m)
                        if mode == "computeonly"
                        else io_bufs.get(m, 2 * chunks.count(m))
                    ),
                )
            )
            for m in sorted(set(chunks))
        }
        work_pool = stack.enter_context(tc.tile_pool(name="work", bufs=workb))
        res_pool = stack.enter_context(tc.tile_pool(name="res", bufs=1))
        # shared sliding scan-output window; scores land at the last
        # NBLK columns (see _emit_scans)
        res = res_pool.tile([P, T + NBLK - 1], f32)
        if mode == "dmaonly":
            nc.vector.memset(res[:, T - 1 : T + NBLK - 1], 0.0)
        io_tiles = ([], []) if mode == "computeonly" else None
        for rep in range(reps):
            _emit_pipeline(
                nc,
                io_pool,
                work_pool,
                res,
                conf,
                acc,
                rep,
                chunks,
                mode,
                io_tiles,
                rings,
                grouped=reps > 1,
            )
        nc.sync.dma_start(out[:], res[:, T - 1 : T + NBLK - 1])
    nc.compile()
    return nc


def make_in_maps(confidences: np.ndarray, accuracies: np.ndarray):
    conf = np.ascontiguousarray(np.asarray(confidences, dtype=np.float32))
    acc = np.ascontiguousarray(np.asarray(accuracies, dtype=np.float32))
    return [
        {
            "confidences": conf[i * ROWS : (i + 1) * ROWS],
            "accuracies": acc[i * ROWS : (i + 1) * ROWS],
        }
        for i in range(N_CORES)
    ]


def reduce_partials(results, accuracies=None) -> np.ndarray:
    # the device scores are complete (t=0 term folded into the scan);
    # just sum, mean, negate
    total = 0.0
    for r in results:
        p = r["partials"].astype(np.float64)
        total += float(p.sum())
    return np.asarray(-(total / B), dtype=np.float32)


def kernel(confidences: np.ndarray, accuracies: np.ndarray) -> np.ndarray:
    if "nc" not in _CACHE:
        _CACHE["nc"] = build_bass()
    nc = _CACHE["nc"]
    results = run_bass_kernel_spmd(
        nc, make_in_maps(confidences, accuracies), list(range(N_CORES))
    ).results
    return reduce_partials(results, accuracies)
